# revision 1
# baseline (speedup 1.0000x reference)
"""Trainium2 Bass kernel for CrossModalAttention (MHA + residual + LayerNorm).

Problem: B=4, L=2048, D=256, H=8, Dh=32.

Dispatch through the axon tunnel costs ~2-4ms of fixed overhead that
GROWS with the number of cores used (per-core RPC round trips), while
per-iteration input bytes pipeline behind execution. Measured A/B over
{8, 4, 2, 1} cores: ONE core minimizes steady-state per-execution time
(~3.1-3.4ms vs ~4.5 for 8 cores), so the whole problem runs on core 0:
the 4 batches stream through a per-batch pipeline (inputs double-
buffered so batch b+1 DMAs overlap batch b compute). All inputs are
packed into ONE flat bf16 blob (~1ms per extra input tensor per
dispatch). Compute in bf16 (4x PE throughput; rel err ~8e-3 vs the
2e-2 gate), LayerNorm statistics in fp32.

Per-batch dataflow (layouts chosen to avoid on-device transposes):
  blob sections per batch: qT/kT/vT [256,2048] (channel-major),
  q_res [2048,256] (token-major, for the residual); shared: weights
  WqT/WkT/WvT/WoT (= W.T so the contraction dim is on partitions),
  bq/bk (added via DVE tensor_scalar on the PSUM->SBUF cast),
  bo' = bo + Wo@bv (bv folded host-side: softmax rows sum to 1), ln.

  QT = WqT.T @ qT [256,2048]; KT likewise; V token-major, interleaved
  with ones blocks per head (vaug) so the PV matmul also produces the
  softmax denominator rows for free.
  scoresT_h = KT_h.T @ QT_h (Dh=32 contraction, 2 heads row-packed per
  2-bank PSUM tile); expS = Exp(scoresT/sqrt(32)) on ScalarE (the
  bottleneck engine: 1 elem/lane/cycle, ~1.18ms of the ~1.39ms exec);
  PV accumulates [ctx; denom] over 16 k-tiles; ctx/denom divide on
  VectorE; out-proj + residual + LayerNorm per 512-row q chunk.
"""

import numpy as np

import concourse.bass as bass
import concourse.tile as tile
from concourse import bacc, mybir
from concourse.bass_utils import run_bass_kernel_spmd

F32 = mybir.dt.float32
BF16 = mybir.dt.bfloat16
D = 256
H = 8
DH = 32
LQ = 2048
LK = 2048
P = 128
SCALE = 1.0 / float(np.sqrt(DH))
LN_EPS = 1e-5
N_CORES = 1
NB = 4 // N_CORES  # batches per core

N_JT = LK // P  # 16 k-token tiles
N_QC = LQ // 512  # 4 q chunks of 512
N_QT = LQ // P  # 16 q token tiles

O_W = 0  # WqT WkT WvT WoT [4*D*D], bq bk [2*D], bo' [D], ln_g [D], ln_b [D]
O_BQK = 4 * D * D
O_BO = O_BQK + 2 * D
O_LNG = O_BO + D
O_LNB = O_LNG + D
O_BAT = O_LNB + D  # per batch: qT, kT, vT [D*LK] each + q_res [LQ*D]
BAT_ELEMS = 4 * D * LK
N_BLOB = O_BAT + NB * BAT_ELEMS


def build_nc():
    nc = bacc.Bacc(None)

    blob_d = nc.declare_dram_parameter("blob", [N_BLOB], BF16, isOutput=False)
    out_d = nc.declare_dram_parameter("out", [NB, LQ, D], BF16, isOutput=True)

    with tile.TileContext(nc) as tc:
        with (
            tc.tile_pool(name="singles", bufs=1) as singles,
            tc.tile_pool(name="bat", bufs=2 if NB > 1 else 1) as bat,
            tc.tile_pool(name="work", bufs=1) as work,
            tc.tile_pool(name="temps", bufs=3) as temps,
            tc.tile_pool(name="mmps", bufs=2, space="PSUM") as mmps,
            tc.tile_pool(name="sps", bufs=2, space="PSUM") as sps,
            tc.tile_pool(name="pvps", bufs=1, space="PSUM") as pvps,
        ):
            # ---- shared constants / weights ------------------------------
            wq_sb = singles.tile([P, 2, D], BF16, tag="wq")
            wk_sb = singles.tile([P, 2, D], BF16, tag="wk")
            wv_sb = singles.tile([P, 2, D], BF16, tag="wv")
            wo_sb = singles.tile([P, 2, D], BF16, tag="wo")
            for i, sb in enumerate((wq_sb, wk_sb, wv_sb, wo_sb)):
                off = O_W + i * D * D
                nc.sync.dma_start(
                    out=sb,
                    in_=blob_d[off : off + D * D].rearrange(
                        "(t p j) -> p t j", t=2, p=P, j=D
                    ),
                )
            bo_row = singles.tile([1, D], BF16, tag="bo_row")
            nc.sync.dma_start(out=bo_row, in_=blob_d[O_BO : O_BO + D][None, :])
            bo_sb = bo_row[:, :]
            bqk_sb = singles.tile([P, 4], BF16, tag="bqk")
            nc.sync.dma_start(
                out=bqk_sb,
                in_=blob_d[O_BQK : O_BQK + 2 * D].rearrange(
                    "(k jt p) -> p (k jt)", k=2, jt=2, p=P
                ),
            )
            bqk_f = singles.tile([P, 4], F32, tag="bqkf")
            nc.vector.tensor_copy(out=bqk_f, in_=bqk_sb)

            ones_sb = singles.tile([1, 512], BF16, tag="ones")
            nc.vector.memset(ones_sb, 1.0)
            eps_sb = singles.tile([P, 1], F32, tag="eps")
            nc.vector.memset(eps_sb, LN_EPS)
            lng_sb = singles.tile([P, D], BF16, tag="lng")
            lnb_sb = singles.tile([P, D], BF16, tag="lnb")
            nc.gpsimd.dma_start(
                out=lng_sb, in_=blob_d[O_LNG : O_LNG + D][None, :].to_broadcast((P, D))
            )
            nc.gpsimd.dma_start(
                out=lnb_sb, in_=blob_d[O_LNB : O_LNB + D][None, :].to_broadcast((P, D))
            )

            # LN stats for all batches; final LN pass once at the end
            mv_all = singles.tile([P, NB, N_QT, 2], F32, tag="mv")
            sd_all = singles.tile([P, NB * N_QT], F32, tag="sd")
            rstd_all = singles.tile([P, NB * N_QT], F32, tag="rstd")
            y_all = [
                singles.tile([P, N_QT, D], BF16, tag=f"y{b}", name=f"y{b}")
                for b in range(NB)
            ]

            for b in range(NB):
                ob = O_BAT + b * BAT_ELEMS
                # ---- per-batch inputs (double-buffered pool) -------------
                xq_sb = bat.tile([P, 2, LQ], BF16, tag="xq")
                xk_sb = bat.tile([P, 2, LK], BF16, tag="xk")
                xv_sb = bat.tile([P, 2, LK], BF16, tag="xv")
                qres_sb = bat.tile([P, N_QT, D], BF16, tag="qres")
                nc.sync.dma_start(
                    out=xq_sb,
                    in_=blob_d[ob : ob + D * LQ].rearrange(
                        "(t p l) -> p t l", t=2, p=P, l=LQ
                    ),
                )
                nc.sync.dma_start(
                    out=xk_sb,
                    in_=blob_d[ob + D * LQ : ob + 2 * D * LK].rearrange(
                        "(t p l) -> p t l", t=2, p=P, l=LK
                    ),
                )
                nc.sync.dma_start(
                    out=xv_sb,
                    in_=blob_d[ob + 2 * D * LK : ob + 3 * D * LK].rearrange(
                        "(t p l) -> p t l", t=2, p=P, l=LK
                    ),
                )
                nc.sync.dma_start(
                    out=qres_sb,
                    in_=blob_d[ob + 3 * D * LK : ob + 4 * D * LK].rearrange(
                        "(t p d) -> p t d", t=N_QT, p=P, d=D
                    ),
                )

                QT_sb = work.tile([P, 2, LQ], BF16, tag="QT")
                KT_sb = work.tile([P, 2, LK], BF16, tag="KT")
                vaug = [
                    work.tile([P, H * 64], BF16, tag=f"vaug{t}", name=f"vaug{t}_{b}")
                    for t in range(N_JT)
                ]
                ctxTn = work.tile([P, 2, LQ], BF16, tag="ctxTn")

                # ---- QKV projections -------------------------------------
                for jt in range(2):
                    for qcc in range(N_QC):
                        ps = mmps.tile([P, 512], F32, tag="mm")
                        nc.tensor.matmul(
                            ps,
                            lhsT=wq_sb[:, 0, jt * P : (jt + 1) * P],
                            rhs=xq_sb[:, 0, qcc * 512 : (qcc + 1) * 512],
                            start=True,
                            stop=False,
                        )
                        nc.tensor.matmul(
                            ps,
                            lhsT=wq_sb[:, 1, jt * P : (jt + 1) * P],
                            rhs=xq_sb[:, 1, qcc * 512 : (qcc + 1) * 512],
                            start=False,
                            stop=True,
                        )
                        nc.vector.tensor_scalar_add(
                            out=QT_sb[:, jt, qcc * 512 : (qcc + 1) * 512],
                            in0=ps,
                            scalar1=bqk_f[:, jt : jt + 1],
                        )
                for jt in range(2):
                    for kc in range(4):
                        ps = mmps.tile([P, 512], F32, tag="mm")
                        nc.tensor.matmul(
                            ps,
                            lhsT=wk_sb[:, 0, jt * P : (jt + 1) * P],
                            rhs=xk_sb[:, 0, kc * 512 : (kc + 1) * 512],
                            start=True,
                            stop=False,
                        )
                        nc.tensor.matmul(
                            ps,
                            lhsT=wk_sb[:, 1, jt * P : (jt + 1) * P],
                            rhs=xk_sb[:, 1, kc * 512 : (kc + 1) * 512],
                            start=False,
                            stop=True,
                        )
                        nc.vector.tensor_scalar_add(
                            out=KT_sb[:, jt, kc * 512 : (kc + 1) * 512],
                            in0=ps,
                            scalar1=bqk_f[:, 2 + jt : 3 + jt],
                        )
                for tt in range(N_JT):
                    ps = mmps.tile([P, D], F32, tag="mm")
                    nc.tensor.matmul(
                        ps,
                        lhsT=xv_sb[:, 0, tt * P : (tt + 1) * P],
                        rhs=wv_sb[:, 0, :],
                        start=True,
                        stop=False,
                    )
                    nc.tensor.matmul(
                        ps,
                        lhsT=xv_sb[:, 1, tt * P : (tt + 1) * P],
                        rhs=wv_sb[:, 1, :],
                        start=False,
                        stop=True,
                    )
                    vt = vaug[tt].rearrange("p (h c) -> p h c", c=64)
                    nc.vector.memset(vt[:, :, DH:], 1.0)
                    nc.vector.tensor_copy(
                        out=vt[:, :, :DH],
                        in_=ps.rearrange("p (h c) -> p h c", c=DH),
                    )

                # ---- attention -------------------------------------------
                for qc in range(N_QC):
                    q0 = qc * 512
                    cu = temps.tile([P, 2, 512], F32, tag="cu")
                    den = temps.tile([P, 2, 512], F32, tag="den")
                    for hp in range(4):
                        pv = pvps.tile([P, 2, 512], F32, tag="pv")
                        for jt in range(N_JT):
                            s = sps.tile([P, 2, 512], F32, tag="s")
                            for e in range(2):
                                h = 2 * hp + e
                                dt = h // 4
                                r0 = (h % 4) * DH
                                nc.tensor.matmul(
                                    s[:, e, :],
                                    lhsT=KT_sb[
                                        r0 : r0 + DH, dt, jt * P : (jt + 1) * P
                                    ],
                                    rhs=QT_sb[r0 : r0 + DH, dt, q0 : q0 + 512],
                                    start=True,
                                    stop=True,
                                    tile_position=(r0, 0),
                                )
                            es = temps.tile([P, 2, 512], BF16, tag="es")
                            nc.scalar.activation(
                                out=es,
                                in_=s,
                                func=mybir.ActivationFunctionType.Exp,
                                scale=SCALE,
                            )
                            for e in range(2):
                                h = 2 * hp + e
                                nc.tensor.matmul(
                                    pv[0:64, e, :],
                                    lhsT=vaug[jt][:, 64 * h : 64 * h + 64],
                                    rhs=es[:, e, :],
                                    start=(jt == 0),
                                    stop=(jt == N_JT - 1),
                                )
                        for e in range(2):
                            h = 2 * hp + e
                            dt = h // 4
                            r0 = (h % 4) * DH
                            nc.vector.tensor_copy(
                                out=cu[r0 : r0 + DH, dt, :], in_=pv[0:DH, e, :]
                            )
                            nc.vector.tensor_copy(
                                out=den[r0 : r0 + DH, dt, :], in_=pv[DH:64, e, :]
                            )
                    rec = temps.tile([P, 2, 512], F32, tag="rec")
                    nc.vector.reciprocal(out=rec, in_=den)
                    for dtv in range(2):
                        nc.vector.tensor_tensor(
                            out=ctxTn[:, dtv, q0 : q0 + 512],
                            in0=cu[:, dtv, :],
                            in1=rec[:, dtv, :],
                            op=mybir.AluOpType.mult,
                        )

                    # ---- output projection + residual --------------------
                    for q4 in range(4):
                        qt = qc * 4 + q4
                        po = mmps.tile([P, D], F32, tag="mm")
                        nc.tensor.matmul(
                            po,
                            lhsT=ctxTn[:, 0, qt * P : (qt + 1) * P],
                            rhs=wo_sb[:, 0, :],
                            start=True,
                            stop=False,
                        )
                        nc.tensor.matmul(
                            po,
                            lhsT=ctxTn[:, 1, qt * P : (qt + 1) * P],
                            rhs=wo_sb[:, 1, :],
                            start=False,
                            stop=False,
                        )
                        nc.tensor.matmul(
                            po,
                            lhsT=ones_sb[:1, :P],
                            rhs=bo_sb,
                            start=False,
                            stop=True,
                        )
                        # y kept bf16 (stats in fp32 via bn_stats on the
                        # bf16 tile: adds ~0.4% rounding, gate is 2e-2)
                        nc.vector.tensor_add(
                            out=y_all[b][:, qt, :], in0=po, in1=qres_sb[:, qt, :]
                        )
                        st = temps.tile([P, 6], F32, tag="st")
                        nc.vector.bn_stats(out=st, in_=y_all[b][:, qt, :])
                        nc.vector.bn_aggr(out=mv_all[:, b, qt, :], in_=st)

            # ---- final LayerNorm pass over all batches -------------------
            nc.scalar.activation(
                out=sd_all,
                in_=mv_all.rearrange("p b q s -> p (b q) s")[:, :, 1:2],
                func=mybir.ActivationFunctionType.Sqrt,
                bias=eps_sb,
            )
            nc.vector.reciprocal(out=rstd_all, in_=sd_all)
            for b in range(NB):
                yo = work.tile([P, N_QT, D], BF16, tag="yo")
                for qt in range(N_QT):
                    i = b * N_QT + qt
                    nc.vector.tensor_scalar(
                        out=yo[:, qt, :],
                        in0=y_all[b][:, qt, :],
                        scalar1=mv_all[:, b, qt, 0:1],
                        scalar2=rstd_all[:, i : i + 1],
                        op0=mybir.AluOpType.subtract,
                        op1=mybir.AluOpType.mult,
                    )
                    nc.vector.tensor_tensor(
                        out=yo[:, qt, :],
                        in0=yo[:, qt, :],
                        in1=lng_sb,
                        op=mybir.AluOpType.mult,
                    )
                    nc.vector.tensor_add(
                        out=yo[:, qt, :], in0=yo[:, qt, :], in1=lnb_sb
                    )
                nc.sync.dma_start(
                    out=out_d[b].rearrange("(t p) d -> p t d", p=P), in_=yo
                )

    nc.finalize()
    return nc


_NC_CACHE = None


def _get_nc():
    global _NC_CACHE
    if _NC_CACHE is None:
        _NC_CACHE = build_nc()
    return _NC_CACHE


def make_in_maps(query, key, value, Wq, bq, Wk, bk, Wv, bv, Wo, bo, ln_g, ln_b):
    import ml_dtypes

    bf = ml_dtypes.bfloat16
    f32 = lambda x: np.asarray(x, dtype=np.float32)

    bo_f = f32(bo) + f32(Wo) @ f32(bv)

    shared = np.empty(O_BAT, dtype=bf)
    for i, W in enumerate((Wq, Wk, Wv, Wo)):
        shared[i * D * D : (i + 1) * D * D] = f32(W).T.astype(bf).ravel()
    shared[O_BQK : O_BQK + D] = f32(bq).astype(bf)
    shared[O_BQK + D : O_BQK + 2 * D] = f32(bk).astype(bf)
    shared[O_BO : O_BO + D] = bo_f.astype(bf)
    shared[O_LNG : O_LNG + D] = f32(ln_g).astype(bf)
    shared[O_LNB : O_LNB + D] = f32(ln_b).astype(bf)

    query = f32(query)
    key = f32(key)
    value = f32(value)
    in_maps = []
    for c in range(N_CORES):
        blob = np.empty(N_BLOB, dtype=bf)
        blob[:O_BAT] = shared
        for j in range(NB):
            b = c * NB + j
            ob = O_BAT + j * BAT_ELEMS
            blob[ob : ob + D * LQ] = (
                np.ascontiguousarray(query[b].T).astype(bf).ravel()
            )
            blob[ob + D * LQ : ob + 2 * D * LK] = (
                np.ascontiguousarray(key[b].T).astype(bf).ravel()
            )
            blob[ob + 2 * D * LK : ob + 3 * D * LK] = (
                np.ascontiguousarray(value[b].T).astype(bf).ravel()
            )
            blob[ob + 3 * D * LK : ob + 4 * D * LK] = query[b].astype(bf).ravel()
        in_maps.append({"blob": blob})
    return in_maps


def kernel(query, key, value, Wq, bq, Wk, bk, Wv, bv, Wo, bo, ln_g, ln_b):
    nc = _get_nc()
    in_maps = make_in_maps(
        query, key, value, Wq, bq, Wk, bk, Wv, bv, Wo, bo, ln_g, ln_b
    )
    res = run_bass_kernel_spmd(nc, in_maps, core_ids=list(range(N_CORES)))
    out = np.empty((4, 2048, 256), dtype=np.float32)
    for c in range(N_CORES):
        o = np.asarray(res.results[c]["out"], dtype=np.float32)
        for j in range(NB):
            out[c * NB + j] = o[j]
    return out



# revision 3
# speedup vs baseline: 1.0604x; 1.0604x over previous
"""Trainium2 Bass kernel for CrossModalAttention (MHA + residual + LayerNorm).

Problem: B=4, L=2048, D=256, H=8, Dh=32.

Dispatch through the axon tunnel costs ~2-4ms of fixed overhead that
GROWS with the number of cores used (per-core RPC round trips), while
per-iteration input bytes pipeline behind execution. Measured A/B over
{8, 4, 2, 1} cores: ONE core minimizes steady-state per-execution time
(~3.1-3.4ms vs ~4.5 for 8 cores), so the whole problem runs on core 0:
the 4 batches stream through a per-batch pipeline (inputs double-
buffered so batch b+1 DMAs overlap batch b compute). All inputs are
packed into ONE flat bf16 blob (~1ms per extra input tensor per
dispatch). Compute in bf16 (4x PE throughput; rel err ~8e-3 vs the
2e-2 gate), LayerNorm statistics in fp32.

Per-batch dataflow (layouts chosen to avoid on-device transposes):
  blob sections per batch: qT/kT/vT [256,2048] (channel-major),
  q_res [2048,256] (token-major, for the residual); shared: weights
  WqT/WkT/WvT/WoT (= W.T so the contraction dim is on partitions),
  bq/bk (added via DVE tensor_scalar on the PSUM->SBUF cast),
  bo' = bo + Wo@bv (bv folded host-side: softmax rows sum to 1), ln.

  QT = WqT.T @ qT [256,2048]; KT likewise; V token-major, interleaved
  with ones blocks per head (vaug) so the PV matmul also produces the
  softmax denominator rows for free.
  scoresT_h = KT_h.T @ QT_h (Dh=32 contraction, 2 heads row-packed per
  2-bank PSUM tile); expS = Exp(scoresT/sqrt(32)) on ScalarE (the
  bottleneck engine: 1 elem/lane/cycle, ~1.18ms of the ~1.39ms exec);
  PV accumulates [ctx; denom] over 16 k-tiles; ctx/denom divide on
  VectorE; out-proj + residual + LayerNorm per 512-row q chunk.
"""

import numpy as np

import concourse.bass as bass
import concourse.tile as tile
from concourse import bacc, mybir
from concourse.bass_utils import run_bass_kernel_spmd

F32 = mybir.dt.float32
BF16 = mybir.dt.bfloat16
I16 = mybir.dt.int16
D = 256
H = 8
DH = 32
LQ = 2048
LK = 2048
P = 128
SCALE = 1.0 / float(np.sqrt(DH))
LN_EPS = 1e-5
N_CORES = 1
NB = 4 // N_CORES  # batches per core

# Schraudolph-style exp on DVE: exp(s*SCALE) ~= bitcast_bf16(int16(
#   s * EXP_A + EXP_B)).  bf16 bits = (exp+127)<<7 | mant7, so
# EXP_A = 128*log2(e)*SCALE and EXP_B = 127*128 + c with c tuned for the
# float->int16 conversion (~3% max rel err; final output err ~3e-3,
# validated through the full pipeline vs the 2e-2 gate).  Offloading a
# fraction of the 134M-element exp from ACT (the bottleneck engine,
# 1 elem/lane/cycle @1.2GHz) to DVE turns the exp wall into a 2-engine
# pipeline.
EXP_A = 128.0 * float(np.log2(np.e)) * SCALE
EXP_B = 16256.0 - 5.0
DVE_JTS = (2, 5, 8, 11, 14)  # 5 of 16 k-tiles exp'd on DVE

N_JT = LK // P  # 16 k-token tiles
N_QC = LQ // 512  # 4 q chunks of 512
N_QT = LQ // P  # 16 q token tiles

O_W = 0  # WqT WkT WvT WoT [4*D*D], bq bk [2*D], bo' [D], ln_g [D], ln_b [D]
O_BQK = 4 * D * D
O_BO = O_BQK + 2 * D
O_LNG = O_BO + D
O_LNB = O_LNG + D
O_BAT = O_LNB + D  # per batch: qT, kT, vT [D*LK] each + q_res [LQ*D]
BAT_ELEMS = 4 * D * LK
N_BLOB = O_BAT + NB * BAT_ELEMS


def build_nc():
    nc = bacc.Bacc(None)

    blob_d = nc.declare_dram_parameter("blob", [N_BLOB], BF16, isOutput=False)
    out_d = nc.declare_dram_parameter("out", [NB, LQ, D], BF16, isOutput=True)

    with tile.TileContext(nc) as tc:
        with (
            tc.tile_pool(name="singles", bufs=1) as singles,
            tc.tile_pool(name="bat", bufs=2 if NB > 1 else 1) as bat,
            tc.tile_pool(name="work", bufs=1) as work,
            tc.tile_pool(name="temps", bufs=3) as temps,
            tc.tile_pool(name="mmps", bufs=2, space="PSUM") as mmps,
            tc.tile_pool(name="sps", bufs=2, space="PSUM") as sps,
            tc.tile_pool(name="pvps", bufs=1, space="PSUM") as pvps,
        ):
            # ---- shared constants / weights ------------------------------
            wq_sb = singles.tile([P, 2, D], BF16, tag="wq")
            wk_sb = singles.tile([P, 2, D], BF16, tag="wk")
            wv_sb = singles.tile([P, 2, D], BF16, tag="wv")
            wo_sb = singles.tile([P, 2, D], BF16, tag="wo")
            for i, sb in enumerate((wq_sb, wk_sb, wv_sb, wo_sb)):
                off = O_W + i * D * D
                nc.sync.dma_start(
                    out=sb,
                    in_=blob_d[off : off + D * D].rearrange(
                        "(t p j) -> p t j", t=2, p=P, j=D
                    ),
                )
            bo_row = singles.tile([1, D], BF16, tag="bo_row")
            nc.sync.dma_start(out=bo_row, in_=blob_d[O_BO : O_BO + D][None, :])
            bo_sb = bo_row[:, :]
            bqk_sb = singles.tile([P, 4], BF16, tag="bqk")
            nc.sync.dma_start(
                out=bqk_sb,
                in_=blob_d[O_BQK : O_BQK + 2 * D].rearrange(
                    "(k jt p) -> p (k jt)", k=2, jt=2, p=P
                ),
            )
            bqk_f = singles.tile([P, 4], F32, tag="bqkf")
            nc.vector.tensor_copy(out=bqk_f, in_=bqk_sb)

            ones_sb = singles.tile([1, 512], BF16, tag="ones")
            nc.vector.memset(ones_sb, 1.0)
            eps_sb = singles.tile([P, 1], F32, tag="eps")
            nc.vector.memset(eps_sb, LN_EPS)
            lng_sb = singles.tile([P, D], BF16, tag="lng")
            lnb_sb = singles.tile([P, D], BF16, tag="lnb")
            nc.gpsimd.dma_start(
                out=lng_sb, in_=blob_d[O_LNG : O_LNG + D][None, :].to_broadcast((P, D))
            )
            nc.gpsimd.dma_start(
                out=lnb_sb, in_=blob_d[O_LNB : O_LNB + D][None, :].to_broadcast((P, D))
            )

            # LN stats for all batches; final LN pass once at the end
            mv_all = singles.tile([P, NB, N_QT, 2], F32, tag="mv")
            sd_all = singles.tile([P, NB * N_QT], F32, tag="sd")
            rstd_all = singles.tile([P, NB * N_QT], F32, tag="rstd")
            y_all = [
                singles.tile([P, N_QT, D], BF16, tag=f"y{b}", name=f"y{b}")
                for b in range(NB)
            ]

            for b in range(NB):
                ob = O_BAT + b * BAT_ELEMS
                # ---- per-batch inputs (double-buffered pool) -------------
                xq_sb = bat.tile([P, 2, LQ], BF16, tag="xq")
                xk_sb = bat.tile([P, 2, LK], BF16, tag="xk")
                xv_sb = bat.tile([P, 2, LK], BF16, tag="xv")
                qres_sb = bat.tile([P, N_QT, D], BF16, tag="qres")
                nc.sync.dma_start(
                    out=xq_sb,
                    in_=blob_d[ob : ob + D * LQ].rearrange(
                        "(t p l) -> p t l", t=2, p=P, l=LQ
                    ),
                )
                nc.sync.dma_start(
                    out=xk_sb,
                    in_=blob_d[ob + D * LQ : ob + 2 * D * LK].rearrange(
                        "(t p l) -> p t l", t=2, p=P, l=LK
                    ),
                )
                nc.sync.dma_start(
                    out=xv_sb,
                    in_=blob_d[ob + 2 * D * LK : ob + 3 * D * LK].rearrange(
                        "(t p l) -> p t l", t=2, p=P, l=LK
                    ),
                )
                nc.sync.dma_start(
                    out=qres_sb,
                    in_=blob_d[ob + 3 * D * LK : ob + 4 * D * LK].rearrange(
                        "(t p d) -> p t d", t=N_QT, p=P, d=D
                    ),
                )

                QT_sb = work.tile([P, 2, LQ], BF16, tag="QT")
                KT_sb = work.tile([P, 2, LK], BF16, tag="KT")
                vaug = [
                    work.tile([P, H * 64], BF16, tag=f"vaug{t}", name=f"vaug{t}_{b}")
                    for t in range(N_JT)
                ]
                ctxTn = work.tile([P, 2, LQ], BF16, tag="ctxTn")

                # ---- QKV projections -------------------------------------
                for jt in range(2):
                    for qcc in range(N_QC):
                        ps = mmps.tile([P, 512], F32, tag="mm")
                        nc.tensor.matmul(
                            ps,
                            lhsT=wq_sb[:, 0, jt * P : (jt + 1) * P],
                            rhs=xq_sb[:, 0, qcc * 512 : (qcc + 1) * 512],
                            start=True,
                            stop=False,
                        )
                        nc.tensor.matmul(
                            ps,
                            lhsT=wq_sb[:, 1, jt * P : (jt + 1) * P],
                            rhs=xq_sb[:, 1, qcc * 512 : (qcc + 1) * 512],
                            start=False,
                            stop=True,
                        )
                        nc.vector.tensor_scalar_add(
                            out=QT_sb[:, jt, qcc * 512 : (qcc + 1) * 512],
                            in0=ps,
                            scalar1=bqk_f[:, jt : jt + 1],
                        )
                for jt in range(2):
                    for kc in range(4):
                        ps = mmps.tile([P, 512], F32, tag="mm")
                        nc.tensor.matmul(
                            ps,
                            lhsT=wk_sb[:, 0, jt * P : (jt + 1) * P],
                            rhs=xk_sb[:, 0, kc * 512 : (kc + 1) * 512],
                            start=True,
                            stop=False,
                        )
                        nc.tensor.matmul(
                            ps,
                            lhsT=wk_sb[:, 1, jt * P : (jt + 1) * P],
                            rhs=xk_sb[:, 1, kc * 512 : (kc + 1) * 512],
                            start=False,
                            stop=True,
                        )
                        nc.vector.tensor_scalar_add(
                            out=KT_sb[:, jt, kc * 512 : (kc + 1) * 512],
                            in0=ps,
                            scalar1=bqk_f[:, 2 + jt : 3 + jt],
                        )
                for tt in range(N_JT):
                    ps = mmps.tile([P, D], F32, tag="mm")
                    nc.tensor.matmul(
                        ps,
                        lhsT=xv_sb[:, 0, tt * P : (tt + 1) * P],
                        rhs=wv_sb[:, 0, :],
                        start=True,
                        stop=False,
                    )
                    nc.tensor.matmul(
                        ps,
                        lhsT=xv_sb[:, 1, tt * P : (tt + 1) * P],
                        rhs=wv_sb[:, 1, :],
                        start=False,
                        stop=True,
                    )
                    vt = vaug[tt].rearrange("p (h c) -> p h c", c=64)
                    nc.vector.memset(vt[:, :, DH:], 1.0)
                    nc.vector.tensor_copy(
                        out=vt[:, :, :DH],
                        in_=ps.rearrange("p (h c) -> p h c", c=DH),
                    )

                # ---- attention -------------------------------------------
                for qc in range(N_QC):
                    q0 = qc * 512
                    cu = temps.tile([P, 2, 512], F32, tag="cu")
                    den = temps.tile([P, 2, 512], F32, tag="den")
                    for hp in range(4):
                        pv = pvps.tile([P, 2, 512], F32, tag="pv")
                        for jt in range(N_JT):
                            s = sps.tile([P, 2, 512], F32, tag="s")
                            for e in range(2):
                                h = 2 * hp + e
                                dt = h // 4
                                r0 = (h % 4) * DH
                                nc.tensor.matmul(
                                    s[:, e, :],
                                    lhsT=KT_sb[
                                        r0 : r0 + DH, dt, jt * P : (jt + 1) * P
                                    ],
                                    rhs=QT_sb[r0 : r0 + DH, dt, q0 : q0 + 512],
                                    start=True,
                                    stop=True,
                                    tile_position=(r0, 0),
                                )
                            if jt in DVE_JTS:
                                esd = temps.tile([P, 2, 512], I16, tag="esd")
                                nc.vector.tensor_scalar(
                                    out=esd,
                                    in0=s,
                                    scalar1=EXP_A,
                                    scalar2=EXP_B,
                                    op0=mybir.AluOpType.mult,
                                    op1=mybir.AluOpType.add,
                                )
                                es = esd.bitcast(BF16)
                            else:
                                es = temps.tile([P, 2, 512], BF16, tag="es")
                                nc.scalar.activation(
                                    out=es,
                                    in_=s,
                                    func=mybir.ActivationFunctionType.Exp,
                                    scale=SCALE,
                                )
                            for e in range(2):
                                h = 2 * hp + e
                                nc.tensor.matmul(
                                    pv[0:64, e, :],
                                    lhsT=vaug[jt][:, 64 * h : 64 * h + 64],
                                    rhs=es[:, e, :],
                                    start=(jt == 0),
                                    stop=(jt == N_JT - 1),
                                )
                        for e in range(2):
                            h = 2 * hp + e
                            dt = h // 4
                            r0 = (h % 4) * DH
                            nc.vector.tensor_copy(
                                out=cu[r0 : r0 + DH, dt, :], in_=pv[0:DH, e, :]
                            )
                            nc.vector.tensor_copy(
                                out=den[r0 : r0 + DH, dt, :], in_=pv[DH:64, e, :]
                            )
                    rec = temps.tile([P, 2, 512], F32, tag="rec")
                    nc.vector.reciprocal(out=rec, in_=den)
                    for dtv in range(2):
                        nc.vector.tensor_tensor(
                            out=ctxTn[:, dtv, q0 : q0 + 512],
                            in0=cu[:, dtv, :],
                            in1=rec[:, dtv, :],
                            op=mybir.AluOpType.mult,
                        )

                    # ---- output projection + residual --------------------
                    for q4 in range(4):
                        qt = qc * 4 + q4
                        po = mmps.tile([P, D], F32, tag="mm")
                        nc.tensor.matmul(
                            po,
                            lhsT=ctxTn[:, 0, qt * P : (qt + 1) * P],
                            rhs=wo_sb[:, 0, :],
                            start=True,
                            stop=False,
                        )
                        nc.tensor.matmul(
                            po,
                            lhsT=ctxTn[:, 1, qt * P : (qt + 1) * P],
                            rhs=wo_sb[:, 1, :],
                            start=False,
                            stop=False,
                        )
                        nc.tensor.matmul(
                            po,
                            lhsT=ones_sb[:1, :P],
                            rhs=bo_sb,
                            start=False,
                            stop=True,
                        )
                        # y kept bf16 (stats in fp32 via bn_stats on the
                        # bf16 tile: adds ~0.4% rounding, gate is 2e-2)
                        nc.vector.tensor_add(
                            out=y_all[b][:, qt, :], in0=po, in1=qres_sb[:, qt, :]
                        )
                        st = temps.tile([P, 6], F32, tag="st")
                        nc.vector.bn_stats(out=st, in_=y_all[b][:, qt, :])
                        nc.vector.bn_aggr(out=mv_all[:, b, qt, :], in_=st)

            # ---- final LayerNorm pass over all batches -------------------
            nc.scalar.activation(
                out=sd_all,
                in_=mv_all.rearrange("p b q s -> p (b q) s")[:, :, 1:2],
                func=mybir.ActivationFunctionType.Sqrt,
                bias=eps_sb,
            )
            nc.vector.reciprocal(out=rstd_all, in_=sd_all)
            for b in range(NB):
                yo = work.tile([P, N_QT, D], BF16, tag="yo")
                for qt in range(N_QT):
                    i = b * N_QT + qt
                    nc.vector.tensor_scalar(
                        out=yo[:, qt, :],
                        in0=y_all[b][:, qt, :],
                        scalar1=mv_all[:, b, qt, 0:1],
                        scalar2=rstd_all[:, i : i + 1],
                        op0=mybir.AluOpType.subtract,
                        op1=mybir.AluOpType.mult,
                    )
                    nc.vector.tensor_tensor(
                        out=yo[:, qt, :],
                        in0=yo[:, qt, :],
                        in1=lng_sb,
                        op=mybir.AluOpType.mult,
                    )
                    nc.vector.tensor_add(
                        out=yo[:, qt, :], in0=yo[:, qt, :], in1=lnb_sb
                    )
                nc.sync.dma_start(
                    out=out_d[b].rearrange("(t p) d -> p t d", p=P), in_=yo
                )

    nc.finalize()
    return nc


_NC_CACHE = None


def _get_nc():
    global _NC_CACHE
    if _NC_CACHE is None:
        _NC_CACHE = build_nc()
    return _NC_CACHE


def make_in_maps(query, key, value, Wq, bq, Wk, bk, Wv, bv, Wo, bo, ln_g, ln_b):
    import ml_dtypes

    bf = ml_dtypes.bfloat16
    f32 = lambda x: np.asarray(x, dtype=np.float32)

    bo_f = f32(bo) + f32(Wo) @ f32(bv)

    shared = np.empty(O_BAT, dtype=bf)
    for i, W in enumerate((Wq, Wk, Wv, Wo)):
        shared[i * D * D : (i + 1) * D * D] = f32(W).T.astype(bf).ravel()
    shared[O_BQK : O_BQK + D] = f32(bq).astype(bf)
    shared[O_BQK + D : O_BQK + 2 * D] = f32(bk).astype(bf)
    shared[O_BO : O_BO + D] = bo_f.astype(bf)
    shared[O_LNG : O_LNG + D] = f32(ln_g).astype(bf)
    shared[O_LNB : O_LNB + D] = f32(ln_b).astype(bf)

    query = f32(query)
    key = f32(key)
    value = f32(value)
    in_maps = []
    for c in range(N_CORES):
        blob = np.empty(N_BLOB, dtype=bf)
        blob[:O_BAT] = shared
        for j in range(NB):
            b = c * NB + j
            ob = O_BAT + j * BAT_ELEMS
            blob[ob : ob + D * LQ] = (
                np.ascontiguousarray(query[b].T).astype(bf).ravel()
            )
            blob[ob + D * LQ : ob + 2 * D * LK] = (
                np.ascontiguousarray(key[b].T).astype(bf).ravel()
            )
            blob[ob + 2 * D * LK : ob + 3 * D * LK] = (
                np.ascontiguousarray(value[b].T).astype(bf).ravel()
            )
            blob[ob + 3 * D * LK : ob + 4 * D * LK] = query[b].astype(bf).ravel()
        in_maps.append({"blob": blob})
    return in_maps


def kernel(query, key, value, Wq, bq, Wk, bk, Wv, bv, Wo, bo, ln_g, ln_b):
    nc = _get_nc()
    in_maps = make_in_maps(
        query, key, value, Wq, bq, Wk, bk, Wv, bv, Wo, bo, ln_g, ln_b
    )
    res = run_bass_kernel_spmd(nc, in_maps, core_ids=list(range(N_CORES)))
    out = np.empty((4, 2048, 256), dtype=np.float32)
    for c in range(N_CORES):
        o = np.asarray(res.results[c]["out"], dtype=np.float32)
        for j in range(NB):
            out[c * NB + j] = o[j]
    return out

